# revision 14
# baseline (speedup 1.0000x reference)
"""Trainium2 Bass kernel for the 2-qubit EstimatorQNN forward pass.

The circuit collapses analytically:
  state after encoding = RY(pi*x0)|0> (x) RY(pi*x1)|0>  (real, rank-1)
  variational layers   = fixed 4x4 unitary U(weights)
  <Z0>                 = s^T A s,  A = Re(U^H Z0 U),  s = a(x0) (x) b(x1)

Expressed in features u=(1, cos(pi x0), sin(pi x0)), v=(1, cos(pi x1), sin(pi x1)):
  out = sum_pq K[p,q] u_p v_q.
Structurally (verified + asserted): K[0,:] == 0 and the 2x2 block K[1:,1:] is
rank-1, so the whole network reduces to THREE cosines:

  out = Ra*cos(pi*x0 - phi_a) + Rb*cos(pi*x0 - phi_b) * cos(pi*x1 - phi_c)

Each cos(pi*(x + d)) with x in [-1,1], d in (-1,1] is computed with the
even/periodic fold  cos(pi*z) = -cos(pi*b),  b = ||z| - 1|  (b in [0,1]),
so the ScalarE Sin activation (valid range [-pi, pi]) evaluates
g = sin(pi*b - pi/2) = -cos(pi*b) in range.

Device op graph per tile (x0, x1 de-interleaved host-side, all contiguous):
  GPSIMD: aa=|x0+da|  ab=|x0+db|  ac=|x1+dc|  bc=|ac-1|
  DVE   : ba=|aa-1|   bb=|ab-1|                       (tensor_scalar, 2x mode)
  ACT   : ga=sin(pi*ba-pi/2)  gb=...  gc=...
  DVE   : p = (gb*Rb)*gc ; y = (ga*Ra)+p              (scalar_tensor_tensor)

The 5 weight-dependent constants are baked as instruction immediates (AP
scalars lower to TensorScalarPtr whose encoding has too few sync-wait slots),
so the program is rebuilt per distinct weight vector; the neuronxcc disk
cache makes repeat compiles for the same weights instant.
"""

import sys

if "/opt/trn_rl_repo" not in sys.path:
    sys.path.insert(0, "/opt/trn_rl_repo")

import numpy as np

import concourse.bass as bass
import concourse.bacc as bacc
import concourse.mybir as mybir
import concourse.tile as tile
from concourse.bass_utils import run_bass_kernel_spmd

N_CORES = 8
B = 4194304
BC = B // N_CORES            # samples per core
P = 128                      # SBUF partitions
H = 1024                     # samples per partition-row per tile
NTILES = BC // (P * H)       # 4

F32 = mybir.dt.float32
PI = float(np.float32(np.pi))
HALF_PI = float(np.float32(np.pi / 2))

_N_QUBITS, _N_LAYERS = 2, 2


# ----------------------------------------------------------------- host math

def _circuit_unitary(w):
    """Fixed 4x4 unitary of the variational layers (float64 complex)."""
    def rx(t):
        c, s = np.cos(t / 2), np.sin(t / 2)
        return np.array([[c, -1j * s], [-1j * s, c]])

    def rz(t):
        c, s = np.cos(t / 2), np.sin(t / 2)
        return np.array([[c - 1j * s, 0], [0, c + 1j * s]])

    def ry(t):
        c, s = np.cos(t / 2), np.sin(t / 2)
        return np.array([[c, -s], [s, c]])

    I2 = np.eye(2)
    CNOT = np.array(
        [[1, 0, 0, 0], [0, 1, 0, 0], [0, 0, 0, 1], [0, 0, 1, 0]], dtype=complex
    )
    U = np.eye(4, dtype=complex)
    off = 0
    for _ in range(_N_LAYERS):
        for q in range(_N_QUBITS):
            for G in (
                rx(w[off + q * 3 + 0]),
                rz(w[off + q * 3 + 1]),
                ry(w[off + q * 3 + 2]),
            ):
                M = np.kron(G, I2) if q == 0 else np.kron(I2, G)
                U = M @ U
        U = CNOT @ U
        off += _N_QUBITS * 3
    return U


def _derive_consts(weights):
    """weights[12] -> (da, db, dc, Ra, Rb) float."""
    w = np.asarray(weights, dtype=np.float64)
    U = _circuit_unitary(w)
    Z0 = np.diag([1.0, 1.0, -1.0, -1.0])
    A = np.real(U.conj().T @ Z0 @ U)

    I2 = np.eye(2)
    Z = np.diag([1.0, -1.0])
    X = np.array([[0.0, 1.0], [1.0, 0.0]])
    Pb = [I2, Z, X]
    K = np.zeros((3, 3))
    for p in range(3):
        for q in range(3):
            acc = 0.0
            for i in range(2):
                for j in range(2):
                    for k in range(2):
                        for l in range(2):
                            acc += A[2 * i + j, 2 * k + l] * Pb[p][i, k] * Pb[q][j, l]
            K[p, q] = 0.25 * acc

    scale = max(np.abs(K).max(), 1e-30)
    assert np.abs(K[0]).max() < 1e-9 * scale, (
        f"structure violated: K row0 nonzero ({K[0]})"
    )

    A, B = float(K[1, 0]), float(K[2, 0])
    r1, r2 = K[1, 1:], K[2, 1:]
    if np.linalg.norm(r1) >= np.linalg.norm(r2):
        n1 = np.linalg.norm(r1)
        phi_c = float(np.arctan2(r1[1], r1[0]))
        P = float(n1)
        Q = float(r2 @ r1) / n1 if n1 > 0 else 0.0
        resid = np.linalg.norm(r2 - (Q / n1) * r1) if n1 > 0 else 0.0
    else:
        n2 = np.linalg.norm(r2)
        phi_c = float(np.arctan2(r2[1], r2[0]))
        Q = float(n2)
        P = float(r1 @ r2) / n2
        resid = np.linalg.norm(r1 - (P / n2) * r2)
    assert resid < 1e-9 * scale, f"structure violated: rank-1 residual {resid}"
    assert np.hypot(P, Q) > 1e-10 * scale, (
        "degenerate weights: cross-term amplitude ~0 (unhandled fast path)"
    )

    dc = -phi_c / np.pi
    dc = float(dc - 2 * np.floor((dc + 1) / 2))  # (-1, 1]

    # y = (P*C0 + Q*S0)*(gc + E) + Fk*W3,  W3 = C0 (orient 0) or S0 (orient 1)
    if abs(Q) >= abs(P):
        orient = 0
        r = P / Q
        Qm = Q
        E = B / Q
        Fk = A - B * P / Q
    else:
        orient = 1
        r = Q / P
        Qm = P
        E = A / P
        Fk = B - A * Q / P
    return (dc, r, Qm, E, Fk, orient)


# ------------------------------------------------------------- device program

def build_program(c6, ntiles=NTILES, h=H):
    """Build the per-core Bass program with the constants as immediates."""
    dc, r, Qm, E, Fk, orient = c6
    dc, r, Qm, E, Fk = (float(np.float32(v)) for v in (dc, r, Qm, E, Fk))

    nc = bacc.Bacc("TRN2", target_bir_lowering=False, debug=False)

    # const APs for Sin biases (float biases lower to const APs)
    for cname, cval in (("neg-half-pi", -HALF_PI), ("pos-half-pi", HALF_PI)):
        t = nc.alloc_sbuf_tensor(f"const-{cname}", [P, 1], F32)
        nc.gpsimd.memset(t.ap(), cval)
        nc.const_aps.aps[(F32, cval)] = t.ap()
    nc.all_engine_barrier()

    # x0 and x1 planes are concatenated along the free dim so each tile is
    # ONE input DMA: 4 in + 4 out = 8 DMAs land on the 8 HWDGE lanes with no
    # same-lane collisions (a second wait overflows the DMA encoding).
    xin = nc.dram_tensor("xin", [ntiles, P, 2 * h], F32, kind="ExternalInput")
    # one output tensor per tile: avoids whole-tensor WAW ordering between
    # the out-DMAs (the DMA instruction encoding only fits one sync wait)
    ys = [
        nc.dram_tensor(f"y{t}", [P, h], F32, kind="ExternalOutput")
        for t in range(ntiles)
    ]

    SIN = mybir.ActivationFunctionType.Sin
    ABS = mybir.ActivationFunctionType.Abs
    ADD = mybir.AluOpType.add
    MULT = mybir.AluOpType.mult
    BAND = mybir.AluOpType.bitwise_and
    U32 = mybir.dt.uint32
    MASK = 0x7FFFFFFF

    with tile.TileContext(nc) as tc:
        with (
            # Pool/ACT/DMA instruction encodings fit only ONE sync wait, so
            # tiles whose writer or reader pattern would add WAR/WAW waits
            # get a dedicated buffer per tile; DVE-written tiles (2 wait
            # slots) cycle through 2 bufs.
            tc.tile_pool(name="xpool", bufs=ntiles) as xpool,
            tc.tile_pool(name="gpool", bufs=ntiles) as gpool,
            tc.tile_pool(name="spool", bufs=ntiles) as spool,
            tc.tile_pool(name="apool", bufs=2) as apool,
            tc.tile_pool(name="wpool", bufs=2) as wpool,
            tc.tile_pool(name="opool", bufs=ntiles) as opool,
        ):
            for t in range(ntiles):
                X = xpool.tile([P, 2 * h], F32, tag="x")
                nc.sync.dma_start(X[:], xin[t])
                X0 = X[:, 0:h]
                X1 = X[:, h:2 * h]

                aE = apool.tile([P, h], F32, tag="aE")
                C0 = apool.tile([P, h], F32, tag="C0")
                S0 = spool.tile([P, h], F32, tag="S0")
                gc = spool.tile([P, h], F32, tag="gc")
                sc = gpool.tile([P, h], F32, tag="sc")
                tc_ = gpool.tile([P, h], F32, tag="tc")
                gq = gpool.tile([P, h], F32, tag="gq")
                ac = wpool.tile([P, h], F32, tag="ac")
                bc = wpool.tile([P, h], F32, tag="bc")
                u2 = wpool.tile([P, h], F32, tag="u2")
                mm = wpool.tile([P, h], F32, tag="mm")
                yy = opool.tile([P, h], F32, tag="yy")

                # x0 side: C0 = cos(pi x0) = sin(pi/2 - pi|x0|), S0 = sin(pi x0)
                nc.scalar.activation(aE[:], X0, ABS)
                nc.scalar.activation(C0[:], aE[:], SIN, bias=HALF_PI, scale=-PI)
                nc.scalar.activation(S0[:], X0, SIN, bias=0.0, scale=PI)
                # x1 side fold: gc = cos(pi*(x1 + dc))
                nc.gpsimd.tensor_scalar(sc[:], X1, dc, None, ADD)
                nc.vector.tensor_scalar(
                    ac[:].bitcast(U32), sc[:].bitcast(U32), MASK, None, BAND
                )
                nc.gpsimd.tensor_scalar(tc_[:], ac[:], -1.0, None, ADD)
                nc.vector.tensor_scalar(
                    bc[:].bitcast(U32), tc_[:].bitcast(U32), MASK, None, BAND
                )
                nc.scalar.activation(gc[:], bc[:], SIN, bias=-HALF_PI, scale=PI)
                nc.gpsimd.tensor_scalar(gq[:], gc[:], E, None, ADD)
                # combine: y = Qm*(r*W1 + W2)*(gc + E) + Fk*W3
                W1, W2 = (C0, S0) if orient == 0 else (S0, C0)
                W3 = C0 if orient == 0 else S0
                nc.vector.scalar_tensor_tensor(u2[:], W1[:], r, W2[:], MULT, ADD)
                nc.vector.scalar_tensor_tensor(mm[:], u2[:], Qm, gq[:], MULT, MULT)
                nc.vector.scalar_tensor_tensor(yy[:], W3[:], Fk, mm[:], MULT, ADD)

                nc.sync.dma_start(ys[t][:], yy[:])

    # Bacc passes redistribute sync waits that overflow the 1-wait
    # instruction encodings into EventSemaphore instructions.
    nc.compile()
    return nc


_PROGRAM_CACHE = {}


def _get_program(c6, ntiles=NTILES, h=H):
    key = (tuple(float(np.float32(v)) for v in c6[:5]), c6[5], ntiles, h)
    if key not in _PROGRAM_CACHE:
        _PROGRAM_CACHE[key] = build_program(c6, ntiles, h)
    return _PROGRAM_CACHE[key]


def make_in_maps(inputs, ntiles=NTILES, h=H, n_cores=N_CORES):
    """Shard full inputs into per-core input maps (host de-interleave)."""
    x = np.asarray(inputs, dtype=np.float32)
    x0 = x[:, 0].reshape(n_cores, ntiles, P, h)
    x1 = x[:, 1].reshape(n_cores, ntiles, P, h)
    xin = np.concatenate([x0, x1], axis=-1)  # [cores, ntiles, P, 2h]
    return [{"xin": xin[i]} for i in range(n_cores)]


def kernel(inputs, weights):
    """Full inputs in, full output out (see module docstring)."""
    c6 = _derive_consts(weights)
    nc = _get_program(c6)
    in_maps = make_in_maps(inputs)
    res = run_bass_kernel_spmd(nc, in_maps, list(range(N_CORES)))
    out = np.concatenate(
        [r[f"y{t}"].reshape(-1) for r in res.results for t in range(NTILES)]
    )
    return out.reshape(B, 1).astype(np.float32)


# revision 16
# speedup vs baseline: 4.9950x; 4.9950x over previous
"""Trainium2 Bass kernel for the 2-qubit EstimatorQNN forward pass.

The circuit collapses analytically:
  state after encoding = RY(pi*x0)|0> (x) RY(pi*x1)|0>  (real, rank-1)
  variational layers   = fixed 4x4 unitary U(weights)
  <Z0>                 = s^T A s,  A = Re(U^H Z0 U),  s = a(x0) (x) b(x1)

Expressed in features u=(1, cos(pi x0), sin(pi x0)), v=(1, cos(pi x1), sin(pi x1)):
  out = sum_pq K[p,q] u_p v_q.
Structurally (verified + asserted): K[0,:] == 0 and the 2x2 block K[1:,1:] is
rank-1, so the whole network reduces to THREE cosines:

  out = Ra*cos(pi*x0 - phi_a) + Rb*cos(pi*x0 - phi_b) * cos(pi*x1 - phi_c)

Each cos(pi*(x + d)) with x in [-1,1], d in (-1,1] is computed with the
even/periodic fold  cos(pi*z) = -cos(pi*b),  b = ||z| - 1|  (b in [0,1]),
so the ScalarE Sin activation (valid range [-pi, pi]) evaluates
g = sin(pi*b - pi/2) = -cos(pi*b) in range.

Device op graph per tile (x0, x1 de-interleaved host-side, all contiguous):
  GPSIMD: aa=|x0+da|  ab=|x0+db|  ac=|x1+dc|  bc=|ac-1|
  DVE   : ba=|aa-1|   bb=|ab-1|                       (tensor_scalar, 2x mode)
  ACT   : ga=sin(pi*ba-pi/2)  gb=...  gc=...
  DVE   : p = (gb*Rb)*gc ; y = (ga*Ra)+p              (scalar_tensor_tensor)

The 5 weight-dependent constants are baked as instruction immediates (AP
scalars lower to TensorScalarPtr whose encoding has too few sync-wait slots),
so the program is rebuilt per distinct weight vector; the neuronxcc disk
cache makes repeat compiles for the same weights instant.
"""

import sys

if "/opt/trn_rl_repo" not in sys.path:
    sys.path.insert(0, "/opt/trn_rl_repo")

import numpy as np

import concourse.bass as bass
import concourse.bacc as bacc
import concourse.mybir as mybir
import concourse.tile as tile
from concourse.bass_utils import run_bass_kernel_spmd

N_CORES = 8
B = 4194304
BC = B // N_CORES            # samples per core
P = 128                      # SBUF partitions
H = 1024                     # samples per partition-row per tile
NTILES = BC // (P * H)       # 4

F32 = mybir.dt.float32
PI = float(np.float32(np.pi))
HALF_PI = float(np.float32(np.pi / 2))

_N_QUBITS, _N_LAYERS = 2, 2


# ----------------------------------------------------------------- host math

def _circuit_unitary(w):
    """Fixed 4x4 unitary of the variational layers (float64 complex)."""
    def rx(t):
        c, s = np.cos(t / 2), np.sin(t / 2)
        return np.array([[c, -1j * s], [-1j * s, c]])

    def rz(t):
        c, s = np.cos(t / 2), np.sin(t / 2)
        return np.array([[c - 1j * s, 0], [0, c + 1j * s]])

    def ry(t):
        c, s = np.cos(t / 2), np.sin(t / 2)
        return np.array([[c, -s], [s, c]])

    I2 = np.eye(2)
    CNOT = np.array(
        [[1, 0, 0, 0], [0, 1, 0, 0], [0, 0, 0, 1], [0, 0, 1, 0]], dtype=complex
    )
    U = np.eye(4, dtype=complex)
    off = 0
    for _ in range(_N_LAYERS):
        for q in range(_N_QUBITS):
            for G in (
                rx(w[off + q * 3 + 0]),
                rz(w[off + q * 3 + 1]),
                ry(w[off + q * 3 + 2]),
            ):
                M = np.kron(G, I2) if q == 0 else np.kron(I2, G)
                U = M @ U
        U = CNOT @ U
        off += _N_QUBITS * 3
    return U


def _derive_consts(weights):
    """weights[12] -> (da, db, dc, Ra, Rb) float."""
    w = np.asarray(weights, dtype=np.float64)
    U = _circuit_unitary(w)
    Z0 = np.diag([1.0, 1.0, -1.0, -1.0])
    A = np.real(U.conj().T @ Z0 @ U)

    I2 = np.eye(2)
    Z = np.diag([1.0, -1.0])
    X = np.array([[0.0, 1.0], [1.0, 0.0]])
    Pb = [I2, Z, X]
    K = np.zeros((3, 3))
    for p in range(3):
        for q in range(3):
            acc = 0.0
            for i in range(2):
                for j in range(2):
                    for k in range(2):
                        for l in range(2):
                            acc += A[2 * i + j, 2 * k + l] * Pb[p][i, k] * Pb[q][j, l]
            K[p, q] = 0.25 * acc

    scale = max(np.abs(K).max(), 1e-30)
    assert np.abs(K[0]).max() < 1e-9 * scale, (
        f"structure violated: K row0 nonzero ({K[0]})"
    )

    A, B = float(K[1, 0]), float(K[2, 0])
    r1, r2 = K[1, 1:], K[2, 1:]
    if np.linalg.norm(r1) >= np.linalg.norm(r2):
        n1 = np.linalg.norm(r1)
        phi_c = float(np.arctan2(r1[1], r1[0]))
        P = float(n1)
        Q = float(r2 @ r1) / n1 if n1 > 0 else 0.0
        resid = np.linalg.norm(r2 - (Q / n1) * r1) if n1 > 0 else 0.0
    else:
        n2 = np.linalg.norm(r2)
        phi_c = float(np.arctan2(r2[1], r2[0]))
        Q = float(n2)
        P = float(r1 @ r2) / n2
        resid = np.linalg.norm(r1 - (P / n2) * r2)
    assert resid < 1e-9 * scale, f"structure violated: rank-1 residual {resid}"
    assert np.hypot(P, Q) > 1e-10 * scale, (
        "degenerate weights: cross-term amplitude ~0 (unhandled fast path)"
    )

    dc = -phi_c / np.pi
    dc = float(dc - 2 * np.floor((dc + 1) / 2))  # (-1, 1]

    # y = (P*C0 + Q*S0)*(gc + E) + Fk*W3,  W3 = C0 (orient 0) or S0 (orient 1)
    if abs(Q) >= abs(P):
        orient = 0
        E = B / Q
        Fk = A - B * P / Q
    else:
        orient = 1
        E = A / P
        Fk = B - A * Q / P
    assert abs(E) < 1e30 and abs(Fk) < 1e30
    return (dc, E, Fk, orient, P, Q)


# ------------------------------------------------------------- device program

def build_program(c6, ntiles=NTILES, h=H):
    """Build the per-core Bass program with the constants as immediates."""
    dc, E, Fk, orient = c6[0], c6[1], c6[2], c6[3]
    dc, E, Fk = (float(np.float32(v)) for v in (dc, E, Fk))

    nc = bacc.Bacc("TRN2", target_bir_lowering=False, debug=False)

    # const APs for activation biases (float biases lower to const APs)
    for cname, cval in (
        ("neg-half-pi", -HALF_PI),
        ("pos-half-pi", HALF_PI),
        ("dc-shift", dc),
    ):
        t = nc.alloc_sbuf_tensor(f"const-{cname}", [P, 1], F32)
        nc.gpsimd.memset(t.ap(), cval)
        nc.const_aps.aps[(F32, cval)] = t.ap()
    nc.all_engine_barrier()

    # x0 and x1 planes are concatenated along the free dim so each tile is
    # ONE input DMA: 4 in + 4 out (+1 weights) DMAs on the 8 HWDGE lanes
    # with no same-lane collisions for the per-tile streams (a second wait
    # overflows the DMA encoding).
    xin = nc.dram_tensor("xin", [ntiles, P, 2 * h], F32, kind="ExternalInput")
    # stationary PE weights [P*I | Q*I]: u2 = P*C0 + Q*S0 via two
    # accumulating identity matmuls; P,Q live in DATA, not the program
    wid = nc.dram_tensor("wid", [P, 2 * P], F32, kind="ExternalInput")
    # one output tensor per tile: avoids whole-tensor WAW ordering between
    # the out-DMAs (the DMA instruction encoding only fits one sync wait)
    ys = [
        nc.dram_tensor(f"y{t}", [P, h], F32, kind="ExternalOutput")
        for t in range(ntiles)
    ]

    SIN = mybir.ActivationFunctionType.Sin
    ABS = mybir.ActivationFunctionType.Abs
    ADD = mybir.AluOpType.add
    MULT = mybir.AluOpType.mult
    BAND = mybir.AluOpType.bitwise_and
    U32 = mybir.dt.uint32
    MASK = 0x7FFFFFFF
    PSUM = bass.MemorySpace.PSUM

    with tile.TileContext(nc) as tc:
        with (
            # Pool/ACT/DMA instruction encodings fit only ONE sync wait, so
            # tiles written by DMA or ACT get a dedicated buffer per tile
            # (no WAR/WAW slot-reuse waits); DVE-written tiles (2 wait
            # slots) cycle through 2 bufs.
            tc.tile_pool(name="cpool", bufs=1) as cpool,
            tc.tile_pool(name="xpool", bufs=ntiles) as xpool,
            tc.tile_pool(name="apool", bufs=ntiles) as apool,
            tc.tile_pool(name="wpool", bufs=2) as wpool,
            tc.tile_pool(name="opool", bufs=ntiles) as opool,
            tc.tile_pool(name="ppool", bufs=2, space=PSUM) as ppool,
        ):
            WI = cpool.tile([P, 2 * P], F32)
            nc.sync.dma_start(WI[:], wid[:])

            for t in range(ntiles):
                X = xpool.tile([P, 2 * h], F32, tag="x")
                nc.sync.dma_start(X[:], xin[t])
                X0 = X[:, 0:h]
                X1 = X[:, h:2 * h]

                ac = apool.tile([P, h], F32, tag="ac")
                gc = apool.tile([P, h], F32, tag="gc")
                C0 = apool.tile([P, h], F32, tag="C0")
                S0 = apool.tile([P, h], F32, tag="S0")
                tc_ = wpool.tile([P, h], F32, tag="tc")
                bc = wpool.tile([P, h], F32, tag="bc")
                aE = wpool.tile([P, h], F32, tag="aE")
                mm = wpool.tile([P, h], F32, tag="mm")
                u2 = ppool.tile([P, h], F32, tag="u2")
                yy = opool.tile([P, h], F32, tag="yy")

                # x1 fold: gc = cos(pi*(x1 + dc)) = sin(pi*||x1+dc|-1| - pi/2)
                nc.scalar.activation(ac[:], X1, ABS, bias=dc, scale=1.0)
                nc.vector.tensor_scalar(tc_[:], ac[:], -1.0, None, ADD)
                nc.vector.tensor_scalar(
                    bc[:].bitcast(U32), tc_[:].bitcast(U32), MASK, None, BAND
                )
                nc.scalar.activation(gc[:], bc[:], SIN, bias=-HALF_PI, scale=PI)
                # x0 side: C0 = cos(pi x0) = sin(pi/2 - pi|x0|), S0 = sin(pi x0)
                nc.vector.tensor_scalar(
                    aE[:].bitcast(U32), X0.bitcast(U32), MASK, None, BAND
                )
                nc.scalar.activation(C0[:], aE[:], SIN, bias=HALF_PI, scale=-PI)
                nc.scalar.activation(S0[:], X0, SIN, bias=0.0, scale=PI)
                # u2 = P*C0 + Q*S0 on the (otherwise idle) tensor engine
                # (512-column chunks: one PSUM bank per matmul)
                for j in range(0, h, 512):
                    cs = slice(j, j + 512)
                    nc.tensor.matmul(
                        u2[:, cs], WI[:, 0:P], C0[:, cs], start=True, stop=False
                    )
                    nc.tensor.matmul(
                        u2[:, cs], WI[:, P:2 * P], S0[:, cs], start=False, stop=True
                    )
                # mm = (gc + E) * u2 ; yy = Fk*W3 + mm
                nc.vector.scalar_tensor_tensor(mm[:], gc[:], E, u2[:], ADD, MULT)
                W3 = C0 if orient == 0 else S0
                nc.vector.scalar_tensor_tensor(yy[:], W3[:], Fk, mm[:], MULT, ADD)

                nc.sync.dma_start(ys[t][:], yy[:])

    # Bacc passes redistribute sync waits that overflow the 1-wait
    # instruction encodings into EventSemaphore instructions.
    nc.compile()
    return nc


_PROGRAM_CACHE = {}


def _get_program(c6, ntiles=NTILES, h=H):
    key = (tuple(float(np.float32(v)) for v in c6[:3]), c6[3], ntiles, h)
    if key not in _PROGRAM_CACHE:
        _PROGRAM_CACHE[key] = build_program(c6, ntiles, h)
    return _PROGRAM_CACHE[key]


def make_in_maps(inputs, PQ, ntiles=NTILES, h=H, n_cores=N_CORES):
    """Shard full inputs into per-core input maps (host de-interleave)."""
    x = np.asarray(inputs, dtype=np.float32)
    x0 = x[:, 0].reshape(n_cores, ntiles, P, h)
    x1 = x[:, 1].reshape(n_cores, ntiles, P, h)
    xin = np.concatenate([x0, x1], axis=-1)  # [cores, ntiles, P, 2h]
    Pc, Qc = PQ
    eye = np.eye(P, dtype=np.float32)
    wid = np.concatenate(
        [np.float32(Pc) * eye, np.float32(Qc) * eye], axis=1
    )
    return [{"xin": xin[i], "wid": wid} for i in range(n_cores)]


def kernel(inputs, weights):
    """Full inputs in, full output out (see module docstring)."""
    c6 = _derive_consts(weights)
    nc = _get_program(c6)
    in_maps = make_in_maps(inputs, c6[4:6])
    res = run_bass_kernel_spmd(nc, in_maps, list(range(N_CORES)))
    out = np.concatenate(
        [r[f"y{t}"].reshape(-1) for r in res.results for t in range(NTILES)]
    )
    return out.reshape(B, 1).astype(np.float32)
